# revision 12
# baseline (speedup 1.0000x reference)
"""ConnectivityLoss kernel for Trainium2 (Bass/Tile), 8-core data-parallel.

Math: the reference's 32-step 3x3 max-dilation chain cancels algebraically.
For binary maps, dilation D(x) >= x pointwise (3x3 SAME window contains the
center), so pred_bin * D32(gt_bin) * gt_bin * D32(pred_bin) == pred_bin * gt_bin
for every pixel: whenever both center bits are 1 the two dilations are 1 at
that pixel too, and otherwise the product is 0 regardless.  Hence

    match[b,k,i,j] = (alpha_pred > t_k) * (alpha_gt > t_k)
                   = (min(alpha_pred, alpha_gt) > t_k)

    err_px = (101 - cnt) / 101      with cnt = #{k in 0..100 : t_k < m},
                                    m = min(alpha_pred, alpha_gt)
    loss   = sum(err_px * [trimap == 128]) / (sum([trimap == 128]) + 1e-8)

cnt is evaluated per pixel with an exact floor trick instead of 101 compares:
    w  = m * mask                     (w = 0 for unmasked pixels, cnt(0) = 0)
    v  = RN(w * S)   with S = fp32(99.99999)  (bits 0x42c7ffff)
    r  = RN(RN(v + 2^23) - 2^23)      -> RNE(v) exactly (magic-number round)
    fl = r - [r > v]                  -> floor(v) exactly
    cnt = fl + [v > 0] + sum_j [w == B7_j]
The 7 constants B7 are the only points of the 2^-23 input lattice (all values
jax.random.uniform can produce) where floor(RN(m*S)) + [m>0] differs from the
true threshold count; the correction makes the formula exhaustively exact over
the entire lattice (verified offline against all 2^23 values).

Sharding: data-parallel over flattened B*H*W pixels, 8 equal contiguous
shards of 32768 = 128x256 elements; each core reduces its shard to
per-partition partial sums (count-sum, mask-sum), host combines the scalars.
"""

import numpy as np

N_CORES = 8
P = 128          # SBUF partitions
F = 256          # free dim; per-core shard = P*F = 32768 pixels
TOTAL = 4 * 1 * 256 * 256

MAGIC = np.float32(8388608.0)        # 2^23
S = np.float32(99.99999)             # bits 0x42c7ffff
B7 = [
    0.49000000953674316,
    0.6000000238418579,
    0.8500000238418579,
    0.8600000143051147,
    0.9700000286102295,
    0.9800000190734863,
    0.9900000095367432,
]

assert int(S.view(np.int32)) == 1120403455

_CACHE = {}


def _build():
    import concourse.bass as bass
    import concourse.tile as tile
    from concourse import mybir

    f32 = mybir.dt.float32
    i32 = mybir.dt.int32
    Op = mybir.AluOpType

    nc = bass.Bass(
        "TRN2",
        target_bir_lowering=False,
        debug=False,
        enable_asserts=True,
        num_devices=N_CORES,
    )
    pred = nc.dram_tensor("pred", [P, F], f32, kind="ExternalInput")
    gt = nc.dram_tensor("gt", [P, F], f32, kind="ExternalInput")
    tri = nc.dram_tensor("tri", [P, F], i32, kind="ExternalInput")
    out_cnt = nc.dram_tensor("cnt_sum", [P, 1], f32, kind="ExternalOutput")
    out_msk = nc.dram_tensor("mask_sum", [P, 1], f32, kind="ExternalOutput")

    with tile.TileContext(nc) as tc:
        with tc.tile_pool(name="pool", bufs=1) as pool:
            tp = pool.tile([P, F], f32)
            tg = pool.tile([P, F], f32)
            tt = pool.tile([P, F], i32)
            nc.gpsimd.dma_start(tp[:], pred[:])
            nc.gpsimd.dma_start(tg[:], gt[:])
            nc.gpsimd.dma_start(tt[:], tri[:])

            mask = pool.tile([P, F], f32)
            sp = pool.tile([P, F], f32)
            sg = pool.tile([P, F], f32)
            v0 = pool.tile([P, F], f32)
            v = pool.tile([P, F], f32)
            t1 = pool.tile([P, F], f32)
            r = pool.tile([P, F], f32)
            g1 = pool.tile([P, F], f32)
            c0 = pool.tile([P, F], f32)
            c1 = pool.tile([P, F], f32)
            sm = pool.tile([P, 1], f32)
            sc = pool.tile([P, 1], f32)

            # HW allows at most 1 sync wait per instruction, and every
            # dependent DVE op already spends it on the DVE self-semaphore.
            # So each DMA'd tile is first touched by an op with NO DVE deps
            # (its one wait = that DMA), and all later ops see the DMA ticks
            # as already-observed.  min commutes with the monotone x->RN(x*S),
            # so scaling before the min is exact:
            #   min(RN(p*S), RN(g*S)) == RN(min(p,g)*S)
            # mask = (tri == 128)
            nc.vector.tensor_scalar(
                mask[:], tt[:], 128.0, None, Op.is_equal
            )
            nc.vector.tensor_scalar(sp[:], tp[:], float(S), None, Op.mult)
            nc.vector.tensor_scalar(sg[:], tg[:], float(S), None, Op.mult)
            nc.vector.tensor_tensor(v0[:], sp[:], sg[:], op=Op.min)
            nc.vector.tensor_tensor(v[:], v0[:], mask[:], op=Op.mult)
            nc.vector.tensor_scalar(t1[:], v[:], float(MAGIC), None, Op.add)
            nc.vector.tensor_scalar(r[:], t1[:], float(MAGIC), None, Op.subtract)
            nc.vector.tensor_tensor(g1[:], r[:], v[:], op=Op.is_gt)
            nc.vector.tensor_tensor(c0[:], r[:], g1[:], op=Op.subtract)
            # c1 = (v > 0) + c0
            nc.vector.scalar_tensor_tensor(
                c1[:], v[:], 0.0, c0[:], op0=Op.is_gt, op1=Op.add
            )
            # lattice corrections in v-space: c = (v == B7[j]*S) + c
            # (each RN(B7[j]*S) is hit by exactly one lattice point: B7[j])
            cur, nxt = c1, c0
            for b in B7:
                vb = float(np.float32(np.float32(b) * S))
                nc.vector.scalar_tensor_tensor(
                    nxt[:], v[:], vb, cur[:], op0=Op.is_equal, op1=Op.add
                )
                cur, nxt = nxt, cur

            nc.vector.tensor_reduce(sc[:], cur[:], mybir.AxisListType.X, Op.add)
            nc.vector.tensor_reduce(sm[:], mask[:], mybir.AxisListType.X, Op.add)

            nc.sync.dma_start(out_cnt[:], sc[:])
            nc.sync.dma_start(out_msk[:], sm[:])

    _split_multi_waits(nc, mybir)
    return nc


def _split_multi_waits(nc, mybir):
    """walrus codegen allows only one sync wait per regular instruction.

    Tile's kernel-tail drain waits on every DMA-queue semaphore plus the
    compute tick at once.  Hoist all but the last wait of any multi-wait
    instruction onto dedicated InstEventSemaphore instructions (which support
    waits) placed immediately before it on the same engine - semantically
    identical, since the engine executes them in order.
    """
    n = 0
    for bb in nc.main_func.blocks:
        new_insts = []
        for ins in bb.instructions:
            si = getattr(ins, "sync_info", None)
            if (
                si is not None
                and si.on_wait
                and len(si.on_wait) > 1
                and not isinstance(ins, mybir.InstEventSemaphore)
            ):
                for wt in si.on_wait[:-1]:
                    ev = mybir.InstEventSemaphore(
                        name=f"waitsplit-{n}", ins=[], outs=[]
                    )
                    n += 1
                    ev.engine = ins.engine
                    ev.sync_info = mybir.SyncInfo(on_wait=[wt], on_update=[])
                    nc.register_instruction(ev, overwrite=True)
                    new_insts.append(ev)
                si.on_wait = si.on_wait[-1:]
            new_insts.append(ins)
        bb.instructions[:] = new_insts


def _get_nc():
    if "nc" not in _CACHE:
        _CACHE["nc"] = _build()
    return _CACHE["nc"]


def _shard(x):
    return np.ascontiguousarray(x.reshape(N_CORES, P, F))


def kernel(alpha_pred, alpha_gt, trimap):
    from concourse.bass_utils import run_bass_kernel_spmd

    ap = np.ascontiguousarray(alpha_pred, dtype=np.float32)
    ag = np.ascontiguousarray(alpha_gt, dtype=np.float32)
    tm = np.ascontiguousarray(trimap, dtype=np.int32)
    assert ap.size == TOTAL and ag.size == TOTAL and tm.size == TOTAL

    aps, ags, tms = _shard(ap), _shard(ag), _shard(tm)
    in_maps = [
        {"pred": aps[i], "gt": ags[i], "tri": tms[i]} for i in range(N_CORES)
    ]

    nc = _get_nc()
    res = run_bass_kernel_spmd(nc, in_maps, list(range(N_CORES))).results

    s_cnt = 0.0
    s_msk = 0.0
    for i in range(N_CORES):
        s_cnt += float(res[i]["cnt_sum"].astype(np.float64).sum())
        s_msk += float(res[i]["mask_sum"].astype(np.float64).sum())

    # loss = sum(mask * (101 - cnt)/101) / (sum(mask) + 1e-8), in fp32 like ref
    num = np.float32((101.0 * s_msk - s_cnt) / 101.0)
    den = np.float32(np.float32(s_msk) + np.float32(1e-8))
    return np.asarray(num / den, dtype=np.float32)


# revision 13
# speedup vs baseline: 1.6503x; 1.6503x over previous
"""ConnectivityLoss kernel for Trainium2 (Bass/Tile), 8-core data-parallel.

Math: the reference's 32-step 3x3 max-dilation chain cancels algebraically.
For binary maps, dilation D(x) >= x pointwise (3x3 SAME window contains the
center), so pred_bin * D32(gt_bin) * gt_bin * D32(pred_bin) == pred_bin * gt_bin
for every pixel: whenever both center bits are 1 the two dilations are 1 at
that pixel too, and otherwise the product is 0 regardless.  Hence

    match[b,k,i,j] = (alpha_pred > t_k) * (alpha_gt > t_k)
                   = (min(alpha_pred, alpha_gt) > t_k)

    err_px = (101 - cnt) / 101      with cnt = #{k in 0..100 : t_k < m},
                                    m = min(alpha_pred, alpha_gt)
    loss   = sum(err_px * [trimap == 128]) / (sum([trimap == 128]) + 1e-8)

cnt is evaluated per pixel with an exactly-rounded two-scale trick instead of
101 compares (t_k = RN(k * fp32(0.01)), the jnp.arange values):

    v128 = m * 128                  exact (power of two)
    v    = v128 * 0.78125           = RN(m * 100)   (0.78125 = 100/128 exact)
    r    = RN(RN(v + 2^23) - 2^23)  = round-to-nearest-int(v), candidate bin
    u    = r * 1.28                 = RN(r * (fp32(0.01)*128)) = t_r * 128
                                      exactly (x128 commutes with rounding)
    cnt  = r + [v128 > u]           since t_r < m <=> u < v128 exactly, and
                                    r is within 0.5 of m*100 so the count can
                                    only be r or r+1.

Verified exhaustively against the reference predicate for ALL 2^30 fp32
values in [0,1): zero mismatches.  m = 0 (unmasked pixels zeroed by the mask
multiply) gives r = 0, u = 0, [0 > 0] = 0 -> cnt = 0 as required.

Sharding: data-parallel over flattened B*H*W pixels, 8 equal contiguous
shards of 32768 = 128x256 elements; each core reduces its shard to
per-partition partial sums (sum r, sum g, sum mask), host combines scalars.
"""

import numpy as np

N_CORES = 8
P = 128          # SBUF partitions
F = 256          # free dim; per-core shard = P*F = 32768 pixels
TOTAL = 4 * 1 * 256 * 256

MAGIC = 8388608.0                       # 2^23
C128 = float(np.float32(0.01) * np.float32(128.0))   # 1.28 in fp32, exact
SCALE = 0.78125                          # 100/128, exact in fp32

_CACHE = {}


def _build():
    import concourse.bass as bass
    import concourse.tile as tile
    from concourse import mybir

    f32 = mybir.dt.float32
    i32 = mybir.dt.int32
    Op = mybir.AluOpType

    nc = bass.Bass(
        "TRN2",
        target_bir_lowering=False,
        debug=False,
        enable_asserts=True,
        num_devices=N_CORES,
    )
    pred = nc.dram_tensor("pred", [P, F], f32, kind="ExternalInput")
    gt = nc.dram_tensor("gt", [P, F], f32, kind="ExternalInput")
    tri = nc.dram_tensor("tri", [P, F], i32, kind="ExternalInput")
    out = nc.dram_tensor("stats", [P, 3], f32, kind="ExternalOutput")

    with tile.TileContext(nc) as tc:
        with tc.tile_pool(name="pool", bufs=1) as pool:
            tp = pool.tile([P, F], f32)
            tg = pool.tile([P, F], f32)
            tt = pool.tile([P, F], i32)
            nc.sync.dma_start(tp[:], pred[:])
            nc.sync.dma_start(tg[:], gt[:])
            nc.sync.dma_start(tt[:], tri[:])

            mask = pool.tile([P, F], f32)
            sp = pool.tile([P, F], f32)
            sg = pool.tile([P, F], f32)
            v0 = pool.tile([P, F], f32)
            v128 = pool.tile([P, F], f32)
            t1 = pool.tile([P, F], f32)
            r = pool.tile([P, F], f32)
            u = pool.tile([P, F], f32)
            g = pool.tile([P, F], f32)
            stats = pool.tile([P, 4], f32)

            # HW allows one sync wait per instruction and dependent DVE ops
            # spend it on the DVE self-semaphore, so each DMA'd tile is first
            # touched by an op with no DVE deps (min commutes with the
            # monotone exact x*128).
            nc.vector.tensor_scalar(mask[:], tt[:], 128.0, None, Op.is_equal)
            nc.vector.tensor_scalar(sp[:], tp[:], 128.0, None, Op.mult)
            nc.vector.tensor_scalar(sg[:], tg[:], 128.0, None, Op.mult)
            nc.vector.tensor_tensor(v0[:], sp[:], sg[:], op=Op.min)
            nc.vector.tensor_tensor(v128[:], v0[:], mask[:], op=Op.mult)
            # t1 = RN(RN(v128*SCALE) + MAGIC); r = t1 - MAGIC  (exact RNE)
            nc.vector.tensor_scalar(t1[:], v128[:], SCALE, MAGIC, Op.mult, Op.add)
            nc.vector.tensor_scalar(r[:], t1[:], MAGIC, None, Op.subtract)
            nc.vector.tensor_scalar(u[:], r[:], C128, None, Op.mult)
            nc.vector.tensor_tensor(g[:], v128[:], u[:], op=Op.is_gt)

            nc.vector.tensor_reduce(stats[:, 0:1], r[:], mybir.AxisListType.X, Op.add)
            nc.vector.tensor_reduce(stats[:, 1:2], g[:], mybir.AxisListType.X, Op.add)
            nc.vector.tensor_reduce(stats[:, 2:3], mask[:], mybir.AxisListType.X, Op.add)

            nc.sync.dma_start(out[:], stats[:, 0:3])

    _split_multi_waits(nc, mybir)
    return nc


def _split_multi_waits(nc, mybir):
    """walrus codegen allows only one sync wait per regular instruction.

    Tile's kernel-tail drain waits on every DMA-queue semaphore plus the
    compute tick at once.  Hoist all but the last wait of any multi-wait
    instruction onto dedicated InstEventSemaphore instructions (which support
    waits) placed immediately before it on the same engine - semantically
    identical, since the engine executes them in order.
    """
    n = 0
    for bb in nc.main_func.blocks:
        new_insts = []
        for ins in bb.instructions:
            si = getattr(ins, "sync_info", None)
            if (
                si is not None
                and si.on_wait
                and len(si.on_wait) > 1
                and not isinstance(ins, mybir.InstEventSemaphore)
            ):
                for wt in si.on_wait[:-1]:
                    ev = mybir.InstEventSemaphore(
                        name=f"waitsplit-{n}", ins=[], outs=[]
                    )
                    n += 1
                    ev.engine = ins.engine
                    ev.sync_info = mybir.SyncInfo(on_wait=[wt], on_update=[])
                    nc.register_instruction(ev, overwrite=True)
                    new_insts.append(ev)
                si.on_wait = si.on_wait[-1:]
            new_insts.append(ins)
        bb.instructions[:] = new_insts


def _get_nc():
    if "nc" not in _CACHE:
        _CACHE["nc"] = _build()
    return _CACHE["nc"]


def _shard(x):
    return np.ascontiguousarray(x.reshape(N_CORES, P, F))


def kernel(alpha_pred, alpha_gt, trimap):
    from concourse.bass_utils import run_bass_kernel_spmd

    ap = np.ascontiguousarray(alpha_pred, dtype=np.float32)
    ag = np.ascontiguousarray(alpha_gt, dtype=np.float32)
    tm = np.ascontiguousarray(trimap, dtype=np.int32)
    assert ap.size == TOTAL and ag.size == TOTAL and tm.size == TOTAL

    aps, ags, tms = _shard(ap), _shard(ag), _shard(tm)
    in_maps = [
        {"pred": aps[i], "gt": ags[i], "tri": tms[i]} for i in range(N_CORES)
    ]

    nc = _get_nc()
    res = run_bass_kernel_spmd(nc, in_maps, list(range(N_CORES))).results

    s_cnt = 0.0
    s_msk = 0.0
    for i in range(N_CORES):
        st = res[i]["stats"].astype(np.float64)
        s_cnt += float(st[:, 0].sum() + st[:, 1].sum())
        s_msk += float(st[:, 2].sum())

    # loss = sum(mask * (101 - cnt)/101) / (sum(mask) + 1e-8), in fp32 like ref
    num = np.float32((101.0 * s_msk - s_cnt) / 101.0)
    den = np.float32(np.float32(s_msk) + np.float32(1e-8))
    return np.asarray(num / den, dtype=np.float32)
